# revision 10
# baseline (speedup 1.0000x reference)
"""Trainium2 Bass kernel: log-odds transform + uniform-grid binning.

Math (per element, bins = linspace(-8, 8, 4096)):
    s   = logit(x) = -ln(1/x - 1),  u = rint(x * 65536) (host u16 cast)
    idx = floor(INVW * s + 2047.5)  INVW = 4095/16
    out = bins[idx]                 (host-side 16KB table decode)

Device chain per unit [128, 2048]:
    DVE : w  = RECIPROCAL_APPROX_FAST(u16)  = 1/u (f32, ~51 ULP, 1x mode)
    ACT : t  = Ln(65536*w - 1) = -s         (f16 out)
    ts  : ob = u16(rne(-INVW*t + 2047))     tensor_scalar
          GPSIMD for units 0..6 (~1.9us each, idle engine), DVE 4x for
          the unit-7 tail chunks.

The DVE recip stream is the critical path (~17us at 1x mode; the 8-slice
custom op has no 2x variant). Everything else is arranged to hang off it
with minimal latency: unit 0 arrives in 512/512/1024-col chunks so the
first recip starts as early as possible; unit 7 leaves in 1024/512/512
chunks so the final Ln->ts->out cascade is short. Per-unit Lns (no
pair-merging) keep ACT latency low; ACT has slack. All DMAs on Sync
HWDGE. run() issues one discarded flush execution first (stale hardware
semaphore safety; the framework epilogue re-zeroes every semaphore, so
the second execution always starts clean).
"""

import numpy as np

import concourse.bacc as bacc
import concourse.mybir as mybir
from concourse import bass_utils
from concourse.dve_ops import RECIP_APPROX_FAST_CONSTS, RECIPROCAL_APPROX_FAST
from concourse.mybir import AluOpType

N = 16_777_216
NCORES = 8
SHARD = N // NCORES
P = 128

NUM_BINS = 4096
INVW = float(np.float32(4095.0 / 16.0))
CADD = 2047.0  # f32->u16 convert is round-to-nearest-even
F32 = mybir.dt.float32
F16 = mybir.dt.float16
U16 = mybir.dt.uint16
Ln = mybir.ActivationFunctionType.Ln

NT = 8
FD = 2048
Q = 512
# chunk layout per unit, in columns (chunks = DMA/compute granularity)
# unit 0 ramps in fine, unit 7 drains out fine, middle units are whole.
CHUNKS = {0: (Q, Q, 2 * Q), 1: (2 * Q, 2 * Q), 7: (2 * Q, Q, Q)}


def build_module(fd=FD, shard=SHARD):
    nt = NT
    assert nt * P * fd == shard
    rc = RECIP_APPROX_FAST_CONSTS

    nc = bacc.Bacc("TRN2", target_bir_lowering=False, debug=False)
    x = nc.dram_tensor("x", [shard], U16, kind="ExternalInput")
    y = nc.dram_tensor("y", [shard], U16, kind="ExternalOutput")
    xv = x[:].rearrange("(n p m) -> n p m", p=P, m=fd)
    yv = y[:].rearrange("(n p m) -> n p m", p=P, m=fd)

    # flat chunk list: (unit, lo, hi, weight) with weight = cols/Q
    def unit_chunks(i):
        cols = CHUNKS.get(i, (fd,))
        lo = 0
        out = []
        for c in cols:
            out.append((i, lo, lo + c, c // Q))
            lo += c
        return out

    in_chunks = [ch for i in range(nt) for ch in unit_chunks(i)]
    # cumulative in_sem threshold (x16 per DMA) keyed by (unit, lo)
    IN_AT = {}
    acc = 0
    for (i, lo, hi, w) in in_chunks:
        acc += 16
        IN_AT[(i, lo)] = acc
    # v1/ln sems count +weight per chunk, cumulative in chunk order
    V1_AT = {}
    acc = 0
    for (i, lo, hi, w) in in_chunks:
        acc += w
        V1_AT[(i, lo)] = acc
    LN_AT = V1_AT  # same chunking and order for Ln

    with (
        nc.sbuf_tensor("xb", [P, nt * fd], U16) as xb,
        nc.sbuf_tensor("wb", [P, nt * fd], F32) as wb,
        nc.sbuf_tensor("tb", [P, nt * fd], F16) as tb,
        nc.sbuf_tensor("ob", [P, nt * fd], U16) as ob,
        nc.sbuf_tensor("warm_in", [P, 1], F32) as warm_in,
        nc.sbuf_tensor("warm_out", [P, 1], F32) as warm_out,
        nc.sbuf_tensor("b_m1", [P, 1], F32) as b_m1,
        nc.semaphore("in_sem") as in_sem,     # +16 per in-DMA
        nc.semaphore("v1_sem") as v1_sem,     # recip: +cols/512 per chunk
        nc.semaphore("ln_sem") as ln_sem,     # Ln: +cols/512 per chunk
        nc.semaphore("v2d_sem") as v2d_sem,   # DVE ts (unit 7): +cols/512
        nc.semaphore("v2g_sem") as v2g_sem,   # GPSIMD ts: +4 per unit
        nc.semaphore("out_sem") as out_sem,   # +16 per out-DMA
        nc.semaphore("misc_sem") as misc_sem,
        nc.Block() as block,
    ):
        def sl(buf, i, lo=0, hi=None):
            s = i * fd
            hi = fd if hi is None else hi
            return buf[:, s + lo:s + hi]

        @block.sync
        def _(sync):
            for (i, lo, hi, w) in in_chunks:
                sync.dma_start(
                    sl(xb, i, lo, hi), xv[i][:, lo:hi]
                ).then_inc(in_sem, 16)
            # outs: units 0..5 whole (after GPSIMD ts); unit 6 and 7a go
            # out on the scalar HWDGE queue (parallel tail dispatch); 7b/7c
            # here.
            for j in range(nt - 2):
                sync.wait_ge(v2g_sem, 4 * (j + 1))
                sync.dma_start(yv[j], sl(ob, j)).then_inc(out_sem, 16)
            d_cnt = 6
            for (i, lo, hi, w) in list(unit_chunks(7))[1:]:
                d_cnt += w
                sync.wait_ge(v2d_sem, d_cnt)
                sync.dma_start(
                    yv[7][:, lo:hi], sl(ob, 7, lo, hi)
                ).then_inc(out_sem, 16)
            # No final out_sem wait: the last out-DMAs complete to DRAM
            # regardless of program end; nothing downstream waits on it.
            sync.sem_clear(v2d_sem)
            sync.sem_clear(v2g_sem)

        @block.scalar
        def _(scalar):
            # Warm the Ln table during the first DMA window.
            scalar.wait_ge(misc_sem, 2)
            nc.scalar.activation(warm_out[:, :], warm_in[:, :], Ln, bias=b_m1[:, :])
            merged = {(2, 0): (2, 0, 2 * fd, 8, 16), (4, 0): (4, 0, 2 * fd, 8, 24)}
            skip = {(3, 0), (5, 0)}
            for (i, lo, hi, w) in in_chunks:
                if (i, lo) in skip:
                    continue
                if (i, lo) in merged:
                    mi, mlo, mhi, mw, thr = merged[(i, lo)]
                    scalar.wait_ge(v1_sem, thr)
                    nc.scalar.activation(
                        sl(tb, mi, mlo, mhi), sl(wb, mi, mlo, mhi),
                        Ln, bias=b_m1[:, :], scale=65536.0,
                    ).then_inc(ln_sem, mw)
                    continue
                scalar.wait_ge(v1_sem, V1_AT[(i, lo)])
                nc.scalar.activation(
                    sl(tb, i, lo, hi), sl(wb, i, lo, hi),
                    Ln, bias=b_m1[:, :], scale=65536.0,
                ).then_inc(ln_sem, w)
            # tail out-DMAs on the scalar HWDGE queue, in parallel with
            # Sync's: unit 6 (after DVE ts6, v2d>=4) and unit 7 chunk a
            # (v2d>=6).
            scalar.wait_ge(v2d_sem, 4)
            nc.scalar.dma_start(yv[6], sl(ob, 6)).then_inc(out_sem, 16)
            c0, c1 = unit_chunks(7)[0][1], unit_chunks(7)[0][2]
            scalar.wait_ge(v2d_sem, 6)
            nc.scalar.dma_start(
                yv[7][:, c0:c1], sl(ob, 7, c0, c1)
            ).then_inc(out_sem, 16)
            scalar.sem_clear(v1_sem)
            scalar.sem_clear(misc_sem)

        @block.vector
        def _(vector):
            nc.vector.memset(warm_in[:, :], 2.0).then_inc(misc_sem, 1)
            nc.vector.memset(b_m1[:, :], -1.0).then_inc(misc_sem, 1)
            for (i, lo, hi, w) in in_chunks:
                vector.wait_ge(in_sem, IN_AT[(i, lo)])
                nc.vector._custom_dve(
                    RECIPROCAL_APPROX_FAST,
                    out=sl(wb, i, lo, hi), in0=sl(xb, i, lo, hi),
                    s0=rc["s0"], s1=rc["s1"], imm2=rc["imm2"],
                ).then_inc(v1_sem, w)
            # tail ts on DVE: unit 6 whole, then unit 7 chunks
            vector.wait_ge(ln_sem, 28)
            nc.vector.tensor_scalar(
                sl(ob, 6), sl(tb, 6),
                -INVW, CADD, AluOpType.mult, AluOpType.add,
            ).then_inc(v2d_sem, 4)
            d_cnt = 4
            for (i, lo, hi, w) in unit_chunks(7):
                d_cnt += w
                vector.wait_ge(ln_sem, LN_AT[(i, lo)])
                nc.vector.tensor_scalar(
                    sl(ob, i, lo, hi), sl(tb, i, lo, hi),
                    -INVW, CADD, AluOpType.mult, AluOpType.add,
                ).then_inc(v2d_sem, w)
            vector.sem_clear(ln_sem)
            vector.sem_clear(in_sem)

        @block.gpsimd
        def _(gpsimd):
            for j in range(nt - 2):
                # all of unit j's Ln chunks done: cumulative weight 4*(j+1)
                gpsimd.wait_ge(ln_sem, 4 * (j + 1))
                nc.gpsimd.tensor_scalar(
                    sl(ob, j), sl(tb, j),
                    -INVW, CADD, AluOpType.mult, AluOpType.add,
                ).then_inc(v2g_sem, 4)

    nc.compile()
    return nc


_module_cache = {}


def _get_module(**kwargs):
    key = repr(sorted(kwargs.items()))
    if key not in _module_cache:
        _module_cache[key] = build_module(**kwargs)
    return _module_cache[key]


def run(Xs, bins, trace=False, **build_kwargs):
    Xs = np.asarray(Xs)
    assert Xs.shape == (N,), Xs.shape
    xin = np.rint(Xs.astype(np.float32) * 65536.0).astype(np.uint16)
    xin = np.ascontiguousarray(xin)
    bins_np = np.asarray(bins, dtype=np.float32)
    nc = _get_module(**build_kwargs)
    shards = xin.reshape(NCORES, SHARD)
    in_maps = [{"x": shards[c]} for c in range(NCORES)]
    # Flush execution: hardware semaphores may hold garbage from a
    # previous (possibly aborted) NEFF; the framework epilogue zeroes
    # every semaphore, so one discarded execution guarantees the real
    # one starts clean.
    bass_utils.run_bass_kernel_spmd(
        nc, in_maps, core_ids=list(range(NCORES)), trace=False
    )
    res = bass_utils.run_bass_kernel_spmd(
        nc, in_maps, core_ids=list(range(NCORES)), trace=trace
    )
    raw = np.concatenate([np.asarray(r["y"]) for r in res.results])
    out = np.take(bins_np, np.minimum(raw, NUM_BINS - 1).astype(np.int64))
    return out.astype(np.float32), res


def kernel(Xs, bins):
    out, _ = run(Xs, bins)
    return out


# revision 11
# speedup vs baseline: 1.1153x; 1.1153x over previous
"""Trainium2 Bass kernel: log-odds transform + uniform-grid binning.

Math (per element, bins = linspace(-8, 8, 4096)):
    s   = logit(x) = -ln(1/x - 1),  u = rint(x * 65536) (host u16 cast)
    idx = floor(INVW * s + 2047.5)  INVW = 4095/16
    out = bins[idx]                 (host-side 16KB table decode)

Device chain per unit [128, 2048]:
    DVE : w  = RECIPROCAL_APPROX_FAST(u16)  = 1/u (f32, ~51 ULP, 1x mode)
    ACT : t  = Ln(65536*w - 1) = -s         (f16 out)
    ts  : ob = u16(rne(-INVW*t + 2047))     tensor_scalar
          GPSIMD for units 0..6 (~1.9us each, idle engine), DVE 4x for
          the unit-7 tail chunks.

The DVE recip stream is the critical path (~17us at 1x mode; the 8-slice
custom op has no 2x variant). Everything else is arranged to hang off it
with minimal latency: unit 0 arrives in 512/512/1024-col chunks so the
first recip starts as early as possible; unit 7 leaves in 1024/512/512
chunks so the final Ln->ts->out cascade is short. Per-unit Lns (no
pair-merging) keep ACT latency low; ACT has slack. All DMAs on Sync
HWDGE. run() issues one discarded flush execution first (stale hardware
semaphore safety; the framework epilogue re-zeroes every semaphore, so
the second execution always starts clean).
"""

import numpy as np

import concourse.bacc as bacc
import concourse.mybir as mybir
from concourse import bass_utils
from concourse.dve_ops import RECIP_APPROX_FAST_CONSTS, RECIPROCAL_APPROX_FAST
from concourse.mybir import AluOpType

N = 16_777_216
NCORES = 8
SHARD = N // NCORES
P = 128

NUM_BINS = 4096
INVW = float(np.float32(4095.0 / 16.0))
CADD = 2047.0  # f32->u16 convert is round-to-nearest-even
F32 = mybir.dt.float32
F16 = mybir.dt.float16
U16 = mybir.dt.uint16
Ln = mybir.ActivationFunctionType.Ln

NT = 8
FD = 2048
Q = 512
# chunk layout per unit, in columns (chunks = DMA/compute granularity)
# unit 0 ramps in fine, unit 7 drains out fine, middle units are whole.
CHUNKS = {0: (Q, Q, 2 * Q), 7: (2 * Q, Q, Q)}


def build_module(fd=FD, shard=SHARD):
    nt = NT
    assert nt * P * fd == shard
    rc = RECIP_APPROX_FAST_CONSTS

    nc = bacc.Bacc("TRN2", target_bir_lowering=False, debug=False)
    x = nc.dram_tensor("x", [shard], U16, kind="ExternalInput")
    y = nc.dram_tensor("y", [shard], U16, kind="ExternalOutput")
    xv = x[:].rearrange("(n p m) -> n p m", p=P, m=fd)
    yv = y[:].rearrange("(n p m) -> n p m", p=P, m=fd)

    # flat chunk list: (unit, lo, hi, weight) with weight = cols/Q
    def unit_chunks(i):
        cols = CHUNKS.get(i, (fd,))
        lo = 0
        out = []
        for c in cols:
            out.append((i, lo, lo + c, c // Q))
            lo += c
        return out

    in_chunks = [ch for i in range(nt) for ch in unit_chunks(i)]
    # cumulative in_sem threshold (x16 per DMA) keyed by (unit, lo)
    IN_AT = {}
    acc = 0
    for (i, lo, hi, w) in in_chunks:
        acc += 16
        IN_AT[(i, lo)] = acc
    # v1/ln sems count +weight per chunk, cumulative in chunk order
    V1_AT = {}
    acc = 0
    for (i, lo, hi, w) in in_chunks:
        acc += w
        V1_AT[(i, lo)] = acc
    LN_AT = V1_AT  # same chunking and order for Ln

    with (
        nc.sbuf_tensor("xb", [P, nt * fd], U16) as xb,
        nc.sbuf_tensor("wb", [P, nt * fd], F32) as wb,
        nc.sbuf_tensor("tb", [P, nt * fd], F16) as tb,
        nc.sbuf_tensor("ob", [P, nt * fd], U16) as ob,
        nc.sbuf_tensor("warm_in", [P, 1], F32) as warm_in,
        nc.sbuf_tensor("warm_out", [P, 1], F32) as warm_out,
        nc.sbuf_tensor("b_m1", [P, 1], F32) as b_m1,
        nc.semaphore("in_sem") as in_sem,     # +16 per in-DMA
        nc.semaphore("v1_sem") as v1_sem,     # recip: +cols/512 per chunk
        nc.semaphore("ln_sem") as ln_sem,     # Ln: +cols/512 per chunk
        nc.semaphore("v2d_sem") as v2d_sem,   # DVE ts (unit 7): +cols/512
        nc.semaphore("v2g_sem") as v2g_sem,   # GPSIMD ts: +4 per unit
        nc.semaphore("out_sem") as out_sem,   # +16 per out-DMA
        nc.semaphore("misc_sem") as misc_sem,
        nc.Block() as block,
    ):
        def sl(buf, i, lo=0, hi=None):
            s = i * fd
            hi = fd if hi is None else hi
            return buf[:, s + lo:s + hi]

        @block.sync
        def _(sync):
            for (i, lo, hi, w) in in_chunks:
                sync.dma_start(
                    sl(xb, i, lo, hi), xv[i][:, lo:hi]
                ).then_inc(in_sem, 16)
            # outs: units 0..5 whole (after GPSIMD ts); unit 6 and 7a go
            # out on the scalar HWDGE queue (parallel tail dispatch); 7b/7c
            # here.
            for j in range(nt - 2):
                sync.wait_ge(v2g_sem, 4 * (j + 1))
                sync.dma_start(yv[j], sl(ob, j)).then_inc(out_sem, 16)
            d_cnt = 6
            for (i, lo, hi, w) in list(unit_chunks(7))[1:]:
                d_cnt += w
                sync.wait_ge(v2d_sem, d_cnt)
                sync.dma_start(
                    yv[7][:, lo:hi], sl(ob, 7, lo, hi)
                ).then_inc(out_sem, 16)
            # No final out_sem wait: the last out-DMAs complete to DRAM
            # regardless of program end; nothing downstream waits on it.
            sync.sem_clear(v2d_sem)
            sync.sem_clear(v2g_sem)

        @block.scalar
        def _(scalar):
            # Warm the Ln table during the first DMA window.
            scalar.wait_ge(misc_sem, 2)
            nc.scalar.activation(warm_out[:, :], warm_in[:, :], Ln, bias=b_m1[:, :])
            for (i, lo, hi, w) in in_chunks:
                scalar.wait_ge(v1_sem, V1_AT[(i, lo)])
                nc.scalar.activation(
                    sl(tb, i, lo, hi), sl(wb, i, lo, hi),
                    Ln, bias=b_m1[:, :], scale=65536.0,
                ).then_inc(ln_sem, w)
            # tail out-DMAs on the scalar HWDGE queue, in parallel with
            # Sync's: unit 6 (after DVE ts6, v2d>=4) and unit 7 chunk a
            # (v2d>=6).
            scalar.wait_ge(v2d_sem, 4)
            nc.scalar.dma_start(yv[6], sl(ob, 6)).then_inc(out_sem, 16)
            c0, c1 = unit_chunks(7)[0][1], unit_chunks(7)[0][2]
            scalar.wait_ge(v2d_sem, 6)
            nc.scalar.dma_start(
                yv[7][:, c0:c1], sl(ob, 7, c0, c1)
            ).then_inc(out_sem, 16)
            scalar.sem_clear(v1_sem)
            scalar.sem_clear(misc_sem)

        @block.vector
        def _(vector):
            nc.vector.memset(warm_in[:, :], 2.0).then_inc(misc_sem, 1)
            nc.vector.memset(b_m1[:, :], -1.0).then_inc(misc_sem, 1)
            for (i, lo, hi, w) in in_chunks:
                vector.wait_ge(in_sem, IN_AT[(i, lo)])
                nc.vector._custom_dve(
                    RECIPROCAL_APPROX_FAST,
                    out=sl(wb, i, lo, hi), in0=sl(xb, i, lo, hi),
                    s0=rc["s0"], s1=rc["s1"], imm2=rc["imm2"],
                ).then_inc(v1_sem, w)
            # tail ts on DVE: unit 6 whole, then unit 7 chunks
            vector.wait_ge(ln_sem, 28)
            nc.vector.tensor_scalar(
                sl(ob, 6), sl(tb, 6),
                -INVW, CADD, AluOpType.mult, AluOpType.add,
            ).then_inc(v2d_sem, 4)
            d_cnt = 4
            for (i, lo, hi, w) in unit_chunks(7):
                d_cnt += w
                vector.wait_ge(ln_sem, LN_AT[(i, lo)])
                nc.vector.tensor_scalar(
                    sl(ob, i, lo, hi), sl(tb, i, lo, hi),
                    -INVW, CADD, AluOpType.mult, AluOpType.add,
                ).then_inc(v2d_sem, w)
            vector.sem_clear(ln_sem)
            vector.sem_clear(in_sem)

        @block.gpsimd
        def _(gpsimd):
            for j in range(nt - 2):
                # all of unit j's Ln chunks done: cumulative weight 4*(j+1)
                gpsimd.wait_ge(ln_sem, 4 * (j + 1))
                nc.gpsimd.tensor_scalar(
                    sl(ob, j), sl(tb, j),
                    -INVW, CADD, AluOpType.mult, AluOpType.add,
                ).then_inc(v2g_sem, 4)

    nc.compile()
    return nc


_module_cache = {}


def _get_module(**kwargs):
    key = repr(sorted(kwargs.items()))
    if key not in _module_cache:
        _module_cache[key] = build_module(**kwargs)
    return _module_cache[key]


def run(Xs, bins, trace=False, **build_kwargs):
    Xs = np.asarray(Xs)
    assert Xs.shape == (N,), Xs.shape
    xin = np.rint(Xs.astype(np.float32) * 65536.0).astype(np.uint16)
    xin = np.ascontiguousarray(xin)
    bins_np = np.asarray(bins, dtype=np.float32)
    nc = _get_module(**build_kwargs)
    shards = xin.reshape(NCORES, SHARD)
    in_maps = [{"x": shards[c]} for c in range(NCORES)]
    # Flush execution: hardware semaphores may hold garbage from a
    # previous (possibly aborted) NEFF; the framework epilogue zeroes
    # every semaphore, so one discarded execution guarantees the real
    # one starts clean.
    bass_utils.run_bass_kernel_spmd(
        nc, in_maps, core_ids=list(range(NCORES)), trace=False
    )
    res = bass_utils.run_bass_kernel_spmd(
        nc, in_maps, core_ids=list(range(NCORES)), trace=trace
    )
    raw = np.concatenate([np.asarray(r["y"]) for r in res.results])
    out = np.take(bins_np, np.minimum(raw, NUM_BINS - 1).astype(np.int64))
    return out.astype(np.float32), res


def kernel(Xs, bins):
    out, _ = run(Xs, bins)
    return out
